# revision 17
# baseline (speedup 1.0000x reference)
"""Causal self-attention (B=2, T=4096, C=1024, H=16) on 8 Trainium2 cores.

Sharding: core c handles batch b = c//4 and heads 4*(c%4)..4*(c%4)+3.
Each core computes q/k/v for its heads, flash-style causal attention
(scores^T formulation, no max subtraction — scores are O(1) bounded),
and a partial output projection over its heads' rows of w_proj. The
host transposes x once per batch, slices weights, sums the four
per-batch projection partials, and assembles k/v.

Matmuls run in float32r (TF32-like, ~1.5e-4 rel err, full PE rate at
free-dim >= 256; fp32 would be 4x slower).
"""
import os

import numpy as np

import concourse.bass as bass
import concourse.tile as tile
from concourse import mybir
from concourse.bass_utils import run_bass_kernel_spmd

F32 = mybir.dt.float32
F32R = mybir.dt.float32r
BF16 = mybir.dt.bfloat16
ATT_DT = BF16   # dtype for scores/PV operands (q/k tiles, P', v tiles)

B, T, C, H, HD = 2, 4096, 1024, 16, 64
NCORES = 8
HPC = H // (NCORES // B)  # heads per core = 4
QKC = HPC * HD            # q (or k) columns per core = 256
CHUNK = 256               # phase-1 token chunk
NCHUNK = T // CHUNK       # 16
IC = 512                  # phase-2 query chunk
NIC = T // IC             # 8
NJT = T // 128            # 32 key tiles
VROW = HPC * (HD + 1) + (128 - HD - 1)  # v_sb free cols: 4*65 + 63 = 323


# ---------------------------------------------------------------------------
# Embedded-wait legalization: this container's walrus rejects sync waits on
# several instruction encodings (Matmult/LDW takes none, most others one).
# Hoist excess waits onto standalone EventSemaphore instructions, the same
# encoding raw-bass wait_ge() emits, inserted just before on the same engine.
_WAIT_BUDGET = {"Matmult": 0, "Ldweights": 0}


def _fix_sync_waits(nc):
    for fn in nc.m.functions:
        for blk in fn.blocks:
            insts = blk.instructions
            idx = 0
            while idx < len(insts):
                inst = insts[idx]
                si = inst.sync_info
                if si is None or not si.on_wait:
                    idx += 1
                    continue
                budget = _WAIT_BUDGET.get(inst.opcode, 1)
                waits = list(si.on_wait)
                if len(waits) <= budget:
                    idx += 1
                    continue
                keep, hoist = waits[:budget], waits[budget:]
                for k, w in enumerate(hoist):
                    ev = mybir.InstEventSemaphore(
                        name=f"{inst.name}_hw{k}",
                        engine=inst.engine,
                        ins=[],
                        outs=[],
                        debug=inst.debug,
                        sync_info=mybir.SyncInfo(on_wait=[w], on_update=[]),
                    )
                    insts.insert(idx, ev)
                    idx += 1
                inst.sync_info = mybir.SyncInfo(
                    on_wait=keep, on_update=list(si.on_update)
                )
                idx += 1


# ---------------------------------------------------------------------------
def build_nc():
    nc = bass.Bass("TRN2", target_bir_lowering=False, debug=False,
                   num_devices=NCORES)
    xT = nc.dram_tensor("xT", [C, T], F32R, kind="ExternalInput").ap()
    wqk = nc.dram_tensor("wqk", [C, 2 * QKC], F32R, kind="ExternalInput").ap()
    wv = nc.dram_tensor("wv", [C, QKC], F32R, kind="ExternalInput").ap()
    wp = nc.dram_tensor("wp", [QKC, C], F32R, kind="ExternalInput").ap()
    bqk = nc.dram_tensor("bqk", [2 * QKC, 1], F32, kind="ExternalInput").ap()
    bv = nc.dram_tensor("bv", [128, QKC], F32, kind="ExternalInput").ap()
    kt_out = nc.dram_tensor("kt_out", [QKC, T], F32, kind="ExternalOutput").ap()
    v_out = nc.dram_tensor("v_out", [T, QKC], F32, kind="ExternalOutput").ap()
    y_out = nc.dram_tensor("y_out", [T, C], F32, kind="ExternalOutput").ap()

    xT_r = xT.rearrange("(kc p) t -> p kc t", p=128)     # [128, 8, T]
    wqk_r = wqk.rearrange("(kc p) m -> p kc m", p=128)   # [128, 8, 512]
    wv_r = wv.rearrange("(kc p) m -> p kc m", p=128)     # [128, 8, 256]
    wp_r = wp.rearrange("(kc p) m -> p kc m", p=128)     # [128, 2, 1024]
    bqk_r = bqk.rearrange("(m p) one -> p (m one)", p=128)  # [128, 4]

    with tile.TileContext(nc) as tc:
        with tc.tile_pool(name="wgt", bufs=1) as wgt, \
             tc.tile_pool(name="big", bufs=1) as bigp, \
             tc.tile_pool(name="xt", bufs=2) as xtp, \
             tc.tile_pool(name="pp", bufs=3) as ppool, \
             tc.tile_pool(name="ep", bufs=2) as ep, \
             tc.tile_pool(name="ps", bufs=1, space="PSUM") as ps:

            # ---- resident tiles -------------------------------------------
            wqk_sb = wgt.tile([128, 8, 2 * QKC], F32R, tag="wqk")
            nc.sync.dma_start(out=wqk_sb, in_=wqk_r)
            wv_sb = wgt.tile([128, 8, QKC], F32R, tag="wv")
            nc.sync.dma_start(out=wv_sb, in_=wv_r)
            bqk_sb = wgt.tile([128, 4], F32, tag="bqk")
            nc.sync.dma_start(out=bqk_sb, in_=bqk_r)
            bv_sb = wgt.tile([128, QKC], F32, tag="bv")
            nc.sync.dma_start(out=bv_sb, in_=bv)

            # padded q tiles: qp[2*hp+par] holds head (2hp+par)'s q rows in
            # its native 64-row band, zeros elsewhere, so scores contract
            # K=128 against the stacked k tile (other head's rows hit zeros).
            qp_sb = [bigp.tile([128, T], ATT_DT, tag=f"qp{i}", name=f"qp{i}")
                     for i in range(4)]
            k_sb = [bigp.tile([128, T], ATT_DT, tag=f"kk{i}", name=f"kk{i}")
                    for i in range(2)]
            v_sb = bigp.tile([128, NJT, VROW], ATT_DT, tag="v", name="v_sb")
            yt_sb = [bigp.tile([128, T], F32R, tag=f"yt{kc}", name=f"yt{kc}")
                     for kc in range(2)]

            # memset cannot write f32r on this walrus; stage consts in f32
            # and tensor_copy (which rounds) into the f32r tiles.
            ones_f = wgt.tile([128, 64], F32, tag="onesf")
            nc.vector.memset(ones_f, 1.0)
            ones_r = wgt.tile([1, 64], F32R, tag="onesr")
            nc.vector.tensor_copy(out=ones_r, in_=ones_f[0:1, :])
            zeros_f = wgt.tile([128, 63], F32, tag="zerosf")
            nc.vector.memset(zeros_f, 0.0)
            for i in range(4):
                dead = qp_sb[i][64:128, :] if i % 2 == 0 else qp_sb[i][0:64, :]
                nc.vector.memset(dead, 0.0)
            # ones column of v (softmax denominator) + zero pad columns
            for h in range(HPC):
                nc.vector.tensor_copy(out=v_sb[:, :, h * (HD + 1) + HD],
                                      in_=ones_f[:, 0:NJT])
            for jt in range(NJT):
                nc.vector.tensor_copy(out=v_sb[:, jt, HPC * (HD + 1):],
                                      in_=zeros_f)

            # ---- phase 1: q^T/k^T (d-major) and v (token-major) -----------
            for ct in range(NCHUNK):
                t0 = ct * CHUNK
                xt = xtp.tile([128, 8, CHUNK], F32R, tag="xt", name=f"xt{ct}")
                nc.sync.dma_start(out=xt, in_=xT_r[:, :, t0:t0 + CHUNK])
                for m in range(4):
                    pq = ps.tile([128, IC], F32, tag=f"ps_y{m % 2}", bufs=1, name=f"pq{ct}_{m}")
                    for kc in range(8):
                        nc.tensor.matmul(
                            pq[:, :CHUNK],
                            wqk_sb[:, kc, m * 128:(m + 1) * 128],
                            xt[:, kc, :],
                            start=(kc == 0), stop=(kc == 7))
                    if m < 2:   # q: write each head's 64-row band
                        for par in range(2):
                            r0 = 64 * par
                            nc.vector.tensor_scalar_add(
                                out=qp_sb[2 * m + par][r0:r0 + 64,
                                                       t0:t0 + CHUNK],
                                in0=pq[r0:r0 + 64, :CHUNK],
                                scalar1=bqk_sb[r0:r0 + 64, m:m + 1])
                    else:
                        nc.vector.tensor_scalar_add(
                            out=k_sb[m - 2][:, t0:t0 + CHUNK],
                            in0=pq[:, :CHUNK],
                            scalar1=bqk_sb[:, m:m + 1])
                    if m >= 2:  # k rows: full-precision copy to HBM output
                        kst = ep.tile([128, CHUNK], F32, tag="kst",
                                      name=f"kst{ct}_{m}")
                        nc.vector.tensor_scalar_add(
                            out=kst, in0=pq[:, :CHUNK],
                            scalar1=bqk_sb[:, m:m + 1])
                        nc.sync.dma_start(
                            out=kt_out[(m - 2) * 128:(m - 1) * 128,
                                       t0:t0 + CHUNK],
                            in_=kst)
                for tt in range(CHUNK // 128):
                    jt = ct * (CHUNK // 128) + tt
                    pv = ps.tile([128, IC], F32, tag=f"ps_y{tt % 2}", bufs=1, name=f"pv{jt}")
                    for kc in range(8):
                        nc.tensor.matmul(
                            pv[:, :QKC],
                            xt[:, kc, tt * 128:(tt + 1) * 128],
                            wv_sb[:, kc, :],
                            start=(kc == 0), stop=(kc == 7))
                    vslice = v_sb[:, jt, :HPC * (HD + 1)].rearrange(
                        "p (h c) -> p h c", c=HD + 1)[:, :, 0:HD]
                    nc.vector.tensor_add(
                        out=vslice,
                        in0=pv[:, :QKC].rearrange("p (h c) -> p h c", c=HD),
                        in1=bv_sb.rearrange("p (h c) -> p h c", c=HD))
                    vst = ep.tile([128, QKC], F32, tag="vst",
                                  name=f"vst{jt}")
                    nc.vector.tensor_add(out=vst, in0=pv[:, :QKC], in1=bv_sb)
                    nc.sync.dma_start(
                        out=v_out[jt * 128:(jt + 1) * 128, :],
                        in_=vst)

            # ---- phase 2: causal attention per (head pair, query chunk) ---
            def emit_epilogue(hp, ic, yps):
                i0 = ic * IC
                rs = []
                for par in range(2):
                    r_sb = ep.tile([1, IC], F32R, tag=f"r{par}",
                                   name=f"r{hp}_{ic}_{par}")
                    with nc.allow_low_precision(reason="softmax denom"):
                        nc.vector.reciprocal(
                            out=r_sb, in_=yps[par][HD:HD + 1, :])
                    rs.append(r_sb)
                for par in range(2):
                    yslice = yt_sb[hp][64 * par:64 * par + 64,
                                       i0:i0 + IC]
                    nc.vector.tensor_copy(out=yslice,
                                          in_=yps[par][0:HD, :])
                    bc = ps.tile([64, IC], F32, tag="ps_b",
                                 name=f"bc{hp}_{ic}_{par}")
                    nc.tensor.matmul(bc, ones_r, rs[par],
                                     start=True, stop=True)
                    nc.vector.tensor_mul(out=yslice, in0=yslice, in1=bc)

            for hp in range(HPC // 2):          # head pairs (0,1), (2,3)
                pend_ep = None   # epilogue runs one query-chunk behind
                for ic in range(NIC):
                    i0 = ic * IC
                    yps = [ps.tile([128, IC], F32, tag=f"ps_y{par}", bufs=1,
                                   name=f"yps{hp}_{ic}_{par}")
                           for par in range(2)]
                    njt = 4 * ic + 4

                    def emit_pv(jt, c0, pts):
                        for par in range(2):
                            h = 2 * hp + par
                            nc.tensor.matmul(
                                yps[par][:, c0:IC],
                                v_sb[:, jt, h * (HD + 1):h * (HD + 1) + 128],
                                pts[par][:, c0:IC],
                                start=(jt == 0), stop=(jt == njt - 1),
                                skip_group_check=True)

                    pending = None  # scores run one j-tile ahead of PV
                    for jt in range(njt):
                        j0 = jt * 128
                        d = jt - 4 * ic  # >= 0 only on diagonal tiles
                        c0 = 128 * d if d > 0 else 0
                        sp = ps.tile([128, 2 * IC], F32, tag="ps_s", bufs=2,
                                     name=f"s{hp}_{ic}_{jt}")
                        for par in range(2):    # heads 2*hp, 2*hp+1
                            nc.tensor.matmul(
                                sp[:, par * IC + c0:(par + 1) * IC],
                                k_sb[hp][:, j0:j0 + 128],
                                qp_sb[2 * hp + par][:, i0 + c0:i0 + IC],
                                start=True, stop=True)
                        ptw = ppool.tile([128, 2, IC], ATT_DT, tag="pt",
                                         name=f"pt{hp}_{ic}_{jt}")
                        sp2 = sp.rearrange("p (a b) -> p a b", a=2)
                        nc.scalar.activation(
                            out=ptw[:, :, c0:IC], in_=sp2[:, :, c0:IC],
                            func=mybir.ActivationFunctionType.Exp,
                            scale=float(1.0 / np.sqrt(HD)))
                        if d >= 0:  # diagonal: zero j>i entries (both heads)
                            nc.gpsimd.affine_select(
                                out=ptw[:, :, c0:IC], in_=ptw[:, :, c0:IC],
                                compare_op=mybir.AluOpType.is_ge,
                                fill=0.0, base=0,
                                pattern=[[0, 2], [1, IC - c0]],
                                channel_multiplier=-1)
                        pts = [ptw[:, 0, :], ptw[:, 1, :]]
                        if pending is not None:
                            emit_pv(*pending)
                        pending = (jt, c0, pts)
                    emit_pv(*pending)
                    if pend_ep is not None:
                        emit_epilogue(*pend_ep)
                    pend_ep = (hp, ic, yps)
                emit_epilogue(*pend_ep)

            # ---- phase 3: partial output projection -----------------------
            wp_sb = wgt.tile([128, 2, C], F32R, tag="wv")  # reuse wv slot
            nc.sync.dma_start(out=wp_sb, in_=wp_r)
            for tt in range(T // 128):
                t0 = tt * 128
                for n in range(2):
                    pj = ps.tile([128, IC], F32, tag=f"ps_y{n}", bufs=1,
                                 name=f"pj{tt}_{n}")
                    for kc in range(2):
                        nc.tensor.matmul(
                            pj,
                            yt_sb[kc][:, t0:t0 + 128],
                            wp_sb[:, kc, n * IC:(n + 1) * IC],
                            start=(kc == 0), stop=(kc == 1))
                    ysb = ep.tile([128, IC], F32, tag="yo", name=f"yo{tt}_{n}")
                    nc.vector.tensor_copy(out=ysb, in_=pj)
                    nc.sync.dma_start(
                        out=y_out[t0:t0 + 128, n * IC:(n + 1) * IC],
                        in_=ysb)
    _fix_sync_waits(nc)
    return nc


_NC_CACHE = None


def kernel(x, w_attn, b_attn, w_proj, b_proj):
    global _NC_CACHE
    x = np.asarray(x, dtype=np.float32)
    w_attn = np.asarray(w_attn, dtype=np.float32)
    b_attn = np.asarray(b_attn, dtype=np.float32)
    w_proj = np.asarray(w_proj, dtype=np.float32)
    b_proj = np.asarray(b_proj, dtype=np.float32)

    xT = [np.ascontiguousarray(x[b].T) for b in range(B)]
    in_maps = []
    for c in range(NCORES):
        b = c // (NCORES // B)
        h0 = HPC * (c % (NCORES // B))
        qcols = np.arange(h0 * HD, (h0 + HPC) * HD)
        in_maps.append({
            "xT": xT[b],
            "wqk": np.ascontiguousarray(
                np.concatenate([w_attn[:, qcols], w_attn[:, C + qcols]],
                               axis=1)),
            "wv": np.ascontiguousarray(w_attn[:, 2 * C + qcols]),
            "wp": np.ascontiguousarray(w_proj[h0 * HD:(h0 + HPC) * HD, :]),
            "bqk": np.ascontiguousarray(
                np.concatenate([b_attn[qcols], b_attn[C + qcols]])
            ).reshape(2 * QKC, 1),
            "bv": np.broadcast_to(b_attn[2 * C + qcols], (128, QKC)).copy(),
        })

    if _NC_CACHE is None:
        _NC_CACHE = build_nc()
    res = run_bass_kernel_spmd(_NC_CACHE, in_maps,
                               core_ids=list(range(NCORES)),
                               trace=bool(os.environ.get("KERNEL_TRACE")))
    global LAST_RESULTS
    LAST_RESULTS = res

    y = np.zeros((B, T, C), dtype=np.float32)
    k = np.zeros((B, H, T, HD), dtype=np.float32)
    v = np.zeros((B, H, T, HD), dtype=np.float32)
    for c in range(NCORES):
        b = c // (NCORES // B)
        h0 = HPC * (c % (NCORES // B))
        r = res.results[c]
        y[b] += r["y_out"]
        # kt_out: [4*64, T] d-major -> (4, T, 64)
        k[b, h0:h0 + HPC] = r["kt_out"].reshape(HPC, HD, T).transpose(0, 2, 1)
        # v_out: [T, 4*64] token-major -> (4, T, 64)
        v[b, h0:h0 + HPC] = r["v_out"].reshape(T, HPC, HD).transpose(1, 0, 2)
    y += b_proj
    return (y, (k, v))


# revision 18
# speedup vs baseline: 1.1072x; 1.1072x over previous
"""Causal self-attention (B=2, T=4096, C=1024, H=16) on 8 Trainium2 cores.

Sharding: core c handles batch b = c//4 and heads 4*(c%4)..4*(c%4)+3.
Each core computes q/k/v for its heads, flash-style causal attention
(scores^T formulation, no max subtraction — scores are O(1) bounded),
and a partial output projection over its heads' rows of w_proj. The
host transposes x once per batch, slices weights, sums the four
per-batch projection partials, and assembles k/v.

Matmuls run in float32r (TF32-like, ~1.5e-4 rel err, full PE rate at
free-dim >= 256; fp32 would be 4x slower).
"""
import os

import numpy as np

import concourse.bass as bass
import concourse.tile as tile
from concourse import mybir
from concourse.bass_utils import run_bass_kernel_spmd

F32 = mybir.dt.float32
F32R = mybir.dt.float32r
BF16 = mybir.dt.bfloat16
ATT_DT = BF16   # dtype for scores/PV operands (q/k tiles, P', v tiles)

B, T, C, H, HD = 2, 4096, 1024, 16, 64
NCORES = 8
HPC = H // (NCORES // B)  # heads per core = 4
QKC = HPC * HD            # q (or k) columns per core = 256
CHUNK = 256               # phase-1 token chunk
NCHUNK = T // CHUNK       # 16
IC = 512                  # phase-2 query chunk
NIC = T // IC             # 8
NJT = T // 128            # 32 key tiles
VROW = HPC * (HD + 1) + (128 - HD - 1)  # v_sb free cols: 4*65 + 63 = 323


# ---------------------------------------------------------------------------
# Embedded-wait legalization: this container's walrus rejects sync waits on
# several instruction encodings (Matmult/LDW takes none, most others one).
# Hoist excess waits onto standalone EventSemaphore instructions, the same
# encoding raw-bass wait_ge() emits, inserted just before on the same engine.
_WAIT_BUDGET = {"Matmult": 0, "Ldweights": 0}


def _fix_sync_waits(nc):
    for fn in nc.m.functions:
        for blk in fn.blocks:
            insts = blk.instructions
            idx = 0
            while idx < len(insts):
                inst = insts[idx]
                si = inst.sync_info
                if si is None or not si.on_wait:
                    idx += 1
                    continue
                budget = _WAIT_BUDGET.get(inst.opcode, 1)
                waits = list(si.on_wait)
                if len(waits) <= budget:
                    idx += 1
                    continue
                keep, hoist = waits[:budget], waits[budget:]
                for k, w in enumerate(hoist):
                    ev = mybir.InstEventSemaphore(
                        name=f"{inst.name}_hw{k}",
                        engine=inst.engine,
                        ins=[],
                        outs=[],
                        debug=inst.debug,
                        sync_info=mybir.SyncInfo(on_wait=[w], on_update=[]),
                    )
                    insts.insert(idx, ev)
                    idx += 1
                inst.sync_info = mybir.SyncInfo(
                    on_wait=keep, on_update=list(si.on_update)
                )
                idx += 1


# ---------------------------------------------------------------------------
def build_nc():
    nc = bass.Bass("TRN2", target_bir_lowering=False, debug=False,
                   num_devices=NCORES)
    xT = nc.dram_tensor("xT", [C, T], F32R, kind="ExternalInput").ap()
    wqk = nc.dram_tensor("wqk", [C, 2 * QKC], F32R, kind="ExternalInput").ap()
    wv = nc.dram_tensor("wv", [C, QKC], F32R, kind="ExternalInput").ap()
    wp = nc.dram_tensor("wp", [QKC, C], F32R, kind="ExternalInput").ap()
    bqk = nc.dram_tensor("bqk", [2 * QKC, 1], F32, kind="ExternalInput").ap()
    bv = nc.dram_tensor("bv", [128, QKC], F32, kind="ExternalInput").ap()
    kt_out = nc.dram_tensor("kt_out", [QKC, T], F32, kind="ExternalOutput").ap()
    v_out = nc.dram_tensor("v_out", [T, QKC], F32, kind="ExternalOutput").ap()
    y_out = nc.dram_tensor("y_out", [T, C], F32, kind="ExternalOutput").ap()

    xT_r = xT.rearrange("(kc p) t -> p kc t", p=128)     # [128, 8, T]
    wqk_r = wqk.rearrange("(kc p) m -> p kc m", p=128)   # [128, 8, 512]
    wv_r = wv.rearrange("(kc p) m -> p kc m", p=128)     # [128, 8, 256]
    wp_r = wp.rearrange("(kc p) m -> p kc m", p=128)     # [128, 2, 1024]
    bqk_r = bqk.rearrange("(m p) one -> p (m one)", p=128)  # [128, 4]

    with tile.TileContext(nc) as tc:
        with tc.tile_pool(name="wgt", bufs=1) as wgt, \
             tc.tile_pool(name="big", bufs=1) as bigp, \
             tc.tile_pool(name="xt", bufs=2) as xtp, \
             tc.tile_pool(name="pp", bufs=3) as ppool, \
             tc.tile_pool(name="ep", bufs=2) as ep, \
             tc.tile_pool(name="ps", bufs=1, space="PSUM") as ps:

            # ---- resident tiles -------------------------------------------
            wqk_sb = wgt.tile([128, 8, 2 * QKC], F32R, tag="wqk")
            nc.sync.dma_start(out=wqk_sb, in_=wqk_r)
            wv_sb = wgt.tile([128, 8, QKC], F32R, tag="wv")
            nc.sync.dma_start(out=wv_sb, in_=wv_r)
            bqk_sb = wgt.tile([128, 4], F32, tag="bqk")
            nc.sync.dma_start(out=bqk_sb, in_=bqk_r)
            bv_sb = wgt.tile([128, QKC], F32, tag="bv")
            nc.sync.dma_start(out=bv_sb, in_=bv)

            # padded q tiles: qp[2*hp+par] holds head (2hp+par)'s q rows in
            # its native 64-row band, zeros elsewhere, so scores contract
            # K=128 against the stacked k tile (other head's rows hit zeros).
            qp_sb = [bigp.tile([128, T], ATT_DT, tag=f"qp{i}", name=f"qp{i}")
                     for i in range(4)]
            k_sb = [bigp.tile([128, T], ATT_DT, tag=f"kk{i}", name=f"kk{i}")
                    for i in range(2)]
            v_sb = bigp.tile([128, NJT, VROW], ATT_DT, tag="v", name="v_sb")
            yt_sb = [bigp.tile([128, T], F32R, tag=f"yt{kc}", name=f"yt{kc}")
                     for kc in range(2)]

            # memset cannot write f32r on this walrus; stage consts in f32
            # and tensor_copy (which rounds) into the f32r tiles.
            ones_f = wgt.tile([128, 64], F32, tag="onesf")
            nc.vector.memset(ones_f, 1.0)
            ones_r = wgt.tile([1, 64], F32R, tag="onesr")
            nc.vector.tensor_copy(out=ones_r, in_=ones_f[0:1, :])
            zeros_f = wgt.tile([128, 63], F32, tag="zerosf")
            nc.vector.memset(zeros_f, 0.0)
            for i in range(4):
                dead = qp_sb[i][64:128, :] if i % 2 == 0 else qp_sb[i][0:64, :]
                nc.vector.memset(dead, 0.0)
            # ones column of v (softmax denominator) + zero pad columns
            for h in range(HPC):
                nc.vector.tensor_copy(out=v_sb[:, :, h * (HD + 1) + HD],
                                      in_=ones_f[:, 0:NJT])
            for jt in range(NJT):
                nc.vector.tensor_copy(out=v_sb[:, jt, HPC * (HD + 1):],
                                      in_=zeros_f)

            # ---- phase 1: q^T/k^T (d-major) and v (token-major) -----------
            for ct in range(NCHUNK):
                t0 = ct * CHUNK
                xt = xtp.tile([128, 8, CHUNK], F32R, tag="xt", name=f"xt{ct}")
                nc.sync.dma_start(out=xt, in_=xT_r[:, :, t0:t0 + CHUNK])
                for m in range(4):
                    pq = ps.tile([128, IC], F32, tag=f"ps_y{m % 2}", bufs=2, name=f"pq{ct}_{m}")
                    for kc in range(8):
                        nc.tensor.matmul(
                            pq[:, :CHUNK],
                            wqk_sb[:, kc, m * 128:(m + 1) * 128],
                            xt[:, kc, :],
                            start=(kc == 0), stop=(kc == 7))
                    if m < 2:   # q: write each head's 64-row band
                        for par in range(2):
                            r0 = 64 * par
                            nc.vector.tensor_scalar_add(
                                out=qp_sb[2 * m + par][r0:r0 + 64,
                                                       t0:t0 + CHUNK],
                                in0=pq[r0:r0 + 64, :CHUNK],
                                scalar1=bqk_sb[r0:r0 + 64, m:m + 1])
                    else:
                        nc.vector.tensor_scalar_add(
                            out=k_sb[m - 2][:, t0:t0 + CHUNK],
                            in0=pq[:, :CHUNK],
                            scalar1=bqk_sb[:, m:m + 1])
                    if m >= 2:  # k rows: full-precision copy to HBM output
                        kst = ep.tile([128, CHUNK], F32, tag="kst",
                                      name=f"kst{ct}_{m}")
                        nc.vector.tensor_scalar_add(
                            out=kst, in0=pq[:, :CHUNK],
                            scalar1=bqk_sb[:, m:m + 1])
                        nc.sync.dma_start(
                            out=kt_out[(m - 2) * 128:(m - 1) * 128,
                                       t0:t0 + CHUNK],
                            in_=kst)
                for tt in range(CHUNK // 128):
                    jt = ct * (CHUNK // 128) + tt
                    pv = ps.tile([128, IC], F32, tag=f"ps_y{tt % 2}", bufs=2, name=f"pv{jt}")
                    for kc in range(8):
                        nc.tensor.matmul(
                            pv[:, :QKC],
                            xt[:, kc, tt * 128:(tt + 1) * 128],
                            wv_sb[:, kc, :],
                            start=(kc == 0), stop=(kc == 7))
                    vslice = v_sb[:, jt, :HPC * (HD + 1)].rearrange(
                        "p (h c) -> p h c", c=HD + 1)[:, :, 0:HD]
                    nc.vector.tensor_add(
                        out=vslice,
                        in0=pv[:, :QKC].rearrange("p (h c) -> p h c", c=HD),
                        in1=bv_sb.rearrange("p (h c) -> p h c", c=HD))
                    vst = ep.tile([128, QKC], F32, tag="vst",
                                  name=f"vst{jt}")
                    nc.vector.tensor_add(out=vst, in0=pv[:, :QKC], in1=bv_sb)
                    nc.sync.dma_start(
                        out=v_out[jt * 128:(jt + 1) * 128, :],
                        in_=vst)

            # ---- phase 2: causal attention per (head pair, query chunk) ---
            def emit_epilogue(hp, ic, yps):
                i0 = ic * IC
                for par in range(2):   # ACT copies the raw yT out first
                    nc.scalar.copy(
                        out=yt_sb[hp][64 * par:64 * par + 64, i0:i0 + IC],
                        in_=yps[par][0:HD, :])
                for par in range(2):
                    r_sb = ep.tile([1, IC], F32R, tag=f"r{par}",
                                   name=f"r{hp}_{ic}_{par}")
                    with nc.allow_low_precision(reason="softmax denom"):
                        nc.vector.reciprocal(
                            out=r_sb, in_=yps[par][HD:HD + 1, :])
                    # broadcast 1/l into the now-free yT rows of the PV bank
                    bc = yps[par][0:64, :]
                    nc.tensor.matmul(bc, ones_r, r_sb,
                                     start=True, stop=True,
                                     skip_group_check=True)
                    yslice = yt_sb[hp][64 * par:64 * par + 64, i0:i0 + IC]
                    nc.vector.tensor_mul(out=yslice, in0=yslice, in1=bc)

            for hp in range(HPC // 2):          # head pairs (0,1), (2,3)
                pend_ep = None   # epilogue runs one query-chunk behind
                for ic in range(NIC):
                    i0 = ic * IC
                    yps = [ps.tile([128, IC], F32, tag=f"ps_y{par}", bufs=2,
                                   name=f"yps{hp}_{ic}_{par}")
                           for par in range(2)]
                    njt = 4 * ic + 4

                    def emit_pv(jt, c0, pts):
                        for par in range(2):
                            h = 2 * hp + par
                            nc.tensor.matmul(
                                yps[par][:, c0:IC],
                                v_sb[:, jt, h * (HD + 1):h * (HD + 1) + 128],
                                pts[par][:, c0:IC],
                                start=(jt == 0), stop=(jt == njt - 1),
                                skip_group_check=True)

                    pending = None  # scores run one j-tile ahead of PV
                    for jt in range(njt):
                        j0 = jt * 128
                        d = jt - 4 * ic  # >= 0 only on diagonal tiles
                        c0 = 128 * d if d > 0 else 0
                        sp = ps.tile([128, 2 * IC], F32, tag="ps_s", bufs=2,
                                     name=f"s{hp}_{ic}_{jt}")
                        for par in range(2):    # heads 2*hp, 2*hp+1
                            nc.tensor.matmul(
                                sp[:, par * IC + c0:(par + 1) * IC],
                                k_sb[hp][:, j0:j0 + 128],
                                qp_sb[2 * hp + par][:, i0 + c0:i0 + IC],
                                start=True, stop=True)
                        ptw = ppool.tile([128, 2, IC], ATT_DT, tag="pt",
                                         name=f"pt{hp}_{ic}_{jt}")
                        sp2 = sp.rearrange("p (a b) -> p a b", a=2)
                        nc.scalar.activation(
                            out=ptw[:, :, c0:IC], in_=sp2[:, :, c0:IC],
                            func=mybir.ActivationFunctionType.Exp,
                            scale=float(1.0 / np.sqrt(HD)))
                        if d >= 0:  # diagonal: zero j>i entries (both heads)
                            nc.gpsimd.affine_select(
                                out=ptw[:, :, c0:IC], in_=ptw[:, :, c0:IC],
                                compare_op=mybir.AluOpType.is_ge,
                                fill=0.0, base=0,
                                pattern=[[0, 2], [1, IC - c0]],
                                channel_multiplier=-1)
                        pts = [ptw[:, 0, :], ptw[:, 1, :]]
                        if pending is not None:
                            emit_pv(*pending)
                        pending = (jt, c0, pts)
                    emit_pv(*pending)
                    if pend_ep is not None:
                        emit_epilogue(*pend_ep)
                    pend_ep = (hp, ic, yps)
                emit_epilogue(*pend_ep)

            # ---- phase 3: partial output projection -----------------------
            wp_sb = wgt.tile([128, 2, C], F32R, tag="wv")  # reuse wv slot
            nc.sync.dma_start(out=wp_sb, in_=wp_r)
            for tt in range(T // 128):
                t0 = tt * 128
                for n in range(2):
                    pj = ps.tile([128, IC], F32, tag=f"ps_y{n}", bufs=2,
                                 name=f"pj{tt}_{n}")
                    for kc in range(2):
                        nc.tensor.matmul(
                            pj,
                            yt_sb[kc][:, t0:t0 + 128],
                            wp_sb[:, kc, n * IC:(n + 1) * IC],
                            start=(kc == 0), stop=(kc == 1))
                    ysb = ep.tile([128, IC], F32, tag="yo", name=f"yo{tt}_{n}")
                    nc.vector.tensor_copy(out=ysb, in_=pj)
                    nc.sync.dma_start(
                        out=y_out[t0:t0 + 128, n * IC:(n + 1) * IC],
                        in_=ysb)
    _fix_sync_waits(nc)
    return nc


_NC_CACHE = None


def kernel(x, w_attn, b_attn, w_proj, b_proj):
    global _NC_CACHE
    x = np.asarray(x, dtype=np.float32)
    w_attn = np.asarray(w_attn, dtype=np.float32)
    b_attn = np.asarray(b_attn, dtype=np.float32)
    w_proj = np.asarray(w_proj, dtype=np.float32)
    b_proj = np.asarray(b_proj, dtype=np.float32)

    xT = [np.ascontiguousarray(x[b].T) for b in range(B)]
    in_maps = []
    for c in range(NCORES):
        b = c // (NCORES // B)
        h0 = HPC * (c % (NCORES // B))
        qcols = np.arange(h0 * HD, (h0 + HPC) * HD)
        in_maps.append({
            "xT": xT[b],
            "wqk": np.ascontiguousarray(
                np.concatenate([w_attn[:, qcols], w_attn[:, C + qcols]],
                               axis=1)),
            "wv": np.ascontiguousarray(w_attn[:, 2 * C + qcols]),
            "wp": np.ascontiguousarray(w_proj[h0 * HD:(h0 + HPC) * HD, :]),
            "bqk": np.ascontiguousarray(
                np.concatenate([b_attn[qcols], b_attn[C + qcols]])
            ).reshape(2 * QKC, 1),
            "bv": np.broadcast_to(b_attn[2 * C + qcols], (128, QKC)).copy(),
        })

    if _NC_CACHE is None:
        _NC_CACHE = build_nc()
    res = run_bass_kernel_spmd(_NC_CACHE, in_maps,
                               core_ids=list(range(NCORES)),
                               trace=bool(os.environ.get("KERNEL_TRACE")))
    global LAST_RESULTS
    LAST_RESULTS = res

    y = np.zeros((B, T, C), dtype=np.float32)
    k = np.zeros((B, H, T, HD), dtype=np.float32)
    v = np.zeros((B, H, T, HD), dtype=np.float32)
    for c in range(NCORES):
        b = c // (NCORES // B)
        h0 = HPC * (c % (NCORES // B))
        r = res.results[c]
        y[b] += r["y_out"]
        # kt_out: [4*64, T] d-major -> (4, T, 64)
        k[b, h0:h0 + HPC] = r["kt_out"].reshape(HPC, HD, T).transpose(0, 2, 1)
        # v_out: [T, 4*64] token-major -> (4, T, 64)
        v[b, h0:h0 + HPC] = r["v_out"].reshape(T, HPC, HD).transpose(1, 0, 2)
    y += b_proj
    return (y, (k, v))
